# revision 14
# baseline (speedup 1.0000x reference)
"""Trainium2 Bass kernel for the CDGR gnn_message_passing module.

Mathematically exact reformulation of the reference (verified to ~4e-4
scale-relative error vs the fp32 jax reference, dominated by float32r
fp22 matmul truncation):

  - softmax rows of A sum to 1  =>  L = I - A, the d-scaling vanishes
  - s2l logits are additively separable in (pixel, node) => the softmax
    over pixels is identical for every node column => app collapses to a
    rank-1 outer product relu(G) (x) softmax(w_in . x)
  - the semantic branch (word attention + 2-layer GCN) is batch
    independent => computed once per core
  - the two chained 1x1 convs fuse: Wlg = final_w[:, :C] @ gw_w

Per batch (2 per core, data-parallel over 8 cores):
  out[o,q] = relu( Wlg @ spiral^T + fa (x) sa + x )  with
  spiral = xv - (E @ xv) / D,  E = exp(S - ub),  S = x_phi @ Dg @ x_phi^T
  computed via S^T tiles (lhsT = M_ext columns) so that E^T column
  slices feed the big E @ xv matmul directly as lhsT, with a fused ones
  column in xv giving D, and a fused K=17 row giving the -ub shift.
"""

import os
from contextlib import ExitStack

import numpy as np

import concourse.bass as bass
import concourse.bacc as bacc
import concourse.mybir as mybir
import concourse.tile as tile
from concourse import masks
from concourse.bass_utils import run_bass_kernel_spmd

FP = mybir.dt.float32
FR = mybir.dt.float32r
AF = mybir.ActivationFunctionType
ALU = mybir.AluOpType

NCORES = 8
BPC = 2          # batches per core
C, HW = 256, 1024
MPHI, NN, DE = 16, 20, 300
KE = DE + 1      # 301 = DEMB + fused-bias row

LAST_EXEC_NS = None
LAST_RESULT = None


def _ksl(total, step=128):
    return [(o, min(step, total - o)) for o in range(0, total, step)]


def _fr(ap):
    return ap.bitcast(FR)


def _mm(nc, out, lhsT, rhs, start, stop):
    nc.tensor.matmul(out, lhsT.bitcast(FR), rhs.bitcast(FR), start=start, stop=stop)


def _build_nc():
    nc = bacc.Bacc()

    def par(name, shape, out=False):
        return nc.declare_dram_parameter(name, list(shape), FP, isOutput=out)

    x_p = par("x", [BPC, C * HW])
    out_p = par("out", [BPC, C * HW], out=True)
    emb_p = par("emb", [NN, DE])
    embTe_p = par("embTe", [KE, NN])          # [emb.T ; ones]
    adj_p = par("adj", [NN, NN])
    wq_p = par("wq", [DE, DE])                # natural
    bq_p = par("bq_col", [DE, 1])
    wk_p = par("wk", [DE, DE])
    bk_p = par("bk_col", [DE, 1])
    wve_p = par("wve", [KE, DE])              # [wv ; bv]
    wo_p = par("wo", [DE, DE])
    bo_p = par("bo_row", [1, DE])
    phiwT_p = par("phi_wT", [C, MPHI])
    phib_p = par("phi_b_col", [MPHI, 1])
    globwT_p = par("glob_wT", [C, MPHI])
    gc1_p = par("gc1_w", [DE, C])
    gc2_p = par("gc2_w", [C, C])
    gww_p = par("gw_w", [C, C])
    win_p = par("w_in_col", [C, 1])
    fwT_p = par("final_wT", [2 * C, C])
    ccol_p = par("const_col", [128, 16])
    crow_p = par("const_row", [1, HW])

    rscr = nc.dram_tensor("rscratch", [BPC, MPHI * HW], FP)

    with tile.TileContext(nc) as tc:
        with nc.allow_low_precision(reason="float32r matmul feed tags"), \
             ExitStack() as ctx:
            _body(ctx, tc, nc, dict(
                x=x_p, out=out_p, emb=emb_p, embTe=embTe_p, adj=adj_p,
                wq=wq_p, bq=bq_p, wk=wk_p, bk=bk_p, wve=wve_p, wo=wo_p, bo=bo_p,
                phiwT=phiwT_p, phib=phib_p, globwT=globwT_p,
                gc1=gc1_p, gc2=gc2_p, gww=gww_p, win=win_p, fwT=fwT_p,
                ccol=ccol_p, crow=crow_p,
                rscr=rscr,
            ))
    nc.finalize()
    return nc


def _body(ctx, tc, nc, P):
    cw = ctx.enter_context(tc.tile_pool(name="cw", bufs=1))       # consts/weights
    sm = ctx.enter_context(tc.tile_pool(name="sm", bufs=2))       # small working
    med = ctx.enter_context(tc.tile_pool(name="med", bufs=2))     # wide, 1 per batch
    big = ctx.enter_context(tc.tile_pool(name="big", bufs=4))     # [128,1024]-ish
    etp = ctx.enter_context(tc.tile_pool(name="etp", bufs=10))    # E^T tiles
    xvp = ctx.enter_context(tc.tile_pool(name="xvp", bufs=14))    # xv_ext tiles
    ps_w = ctx.enter_context(tc.tile_pool(name="ps_w", bufs=2, space="PSUM"))
    ps_x = ctx.enter_context(tc.tile_pool(name="ps_x", bufs=2, space="PSUM"))
    ps_t = ctx.enter_context(tc.tile_pool(name="ps_t", bufs=2, space="PSUM"))

    def load_w(dram, k, n, tag, fr=True):
        """DRAM [k, n] -> list of SBUF tiles [<=128, n] along k."""
        ts = []
        for i, (o, s) in enumerate(_ksl(k)):
            t = cw.tile([s, n], FP, tag=f"{tag}{i}")
            if fr:
                nc.sync.dma_start(_fr(t[:]), _fr(dram[o:o + s, :]))
            else:
                nc.sync.dma_start(t[:], dram[o:o + s, :])
            ts.append(t)
        return ts

    ident = cw.tile([128, 128], FP, tag="ident")
    masks.make_identity(nc, ident[:])

    embTe = load_w(P["embTe"][:], KE, NN, "embTe")
    emb_sb = load_w(P["emb"][:], NN, DE, "emb", fr=False)[0]
    adj_sb = load_w(P["adj"][:], NN, NN, "adj", fr=False)[0]
    wq = load_w(P["wq"][:], DE, DE, "wq")
    wk = load_w(P["wk"][:], DE, DE, "wk")
    bq = load_w(P["bq"][:], DE, 1, "bq", fr=False)
    bk = load_w(P["bk"][:], DE, 1, "bk", fr=False)
    wve = load_w(P["wve"][:], KE, DE, "wve")
    wo = load_w(P["wo"][:], DE, DE, "wo")
    bo_row = load_w(P["bo"][:], 1, DE, "bo_row", fr=False)[0]
    phiwT = load_w(P["phiwT"][:], C, MPHI, "phiwT")
    phib = load_w(P["phib"][:], MPHI, 1, "phib", fr=False)[0]
    globwT = load_w(P["globwT"][:], C, MPHI, "globwT")
    gc1 = load_w(P["gc1"][:], DE, C, "gc1")
    gc2 = load_w(P["gc2"][:], C, C, "gc2")
    gww = load_w(P["gww"][:], C, C, "gww")
    win = load_w(P["win"][:], C, 1, "win")
    fwT = load_w(P["fwT"][:], 2 * C, C, "fwT")

    one_row = cw.tile([1, NN], FP, tag="one_row")
    nc.sync.dma_start(_fr(one_row[:]), _fr(P["crow"][0:1, 0:NN]))
    ones20 = cw.tile([NN, 8], FP, tag="ones20")
    nc.sync.dma_start(_fr(ones20[:]), _fr(P["ccol"][0:NN, 0:8]))
    inv20 = cw.tile([NN, 8], FP, tag="inv20")
    nc.sync.dma_start(_fr(inv20[:]), _fr(P["ccol"][0:NN, 8:16]))
    onescol = cw.tile([128, 8], FP, tag="onescol")
    nc.sync.dma_start(onescol[:], P["ccol"][:, 0:8])

    # ---------------- semantic branch (batch independent) ----------------
    # qT, kT [300, 20] in 3 partition chunks: qT = wq^T @ emb^T (+ bias col)
    def qt_like(w, bcol, tag):
        outs = []
        for mi, (mo, ms) in enumerate(_ksl(DE)):
            ps = ps_t.tile([ms, NN], FP, tag="ps_t")
            for ki, (ko, ks) in enumerate(_ksl(DE)):
                _mm(nc, ps[:], w[ki][:, mo:mo + ms], embTe[ki][0:ks, :],
                    start=(ki == 0), stop=(ki == 2))
            t = sm.tile([ms, NN], FP, tag=f"{tag}{mi}")
            nc.scalar.activation(_fr(t[:]), ps[:], AF.Identity, bias=bcol[mi][:, 0:1])
            outs.append(t)
        return outs

    qT = qt_like(wq, [bq[0], bq[1], bq[2]], "qT")
    kT = qt_like(wk, [bk[0], bk[1], bk[2]], "kT")

    # v natural [20, 300] = embTe.T @ wve (bias row fused)
    ps = ps_t.tile([NN, DE], FP, tag="ps_t")
    for ki, (ko, ks) in enumerate(_ksl(KE)):
        _mm(nc, ps[:], embTe[ki][:, :], wve[ki][:, :], start=(ki == 0), stop=(ki == 2))
    v_sb = sm.tile([NN, DE], FP, tag="v_sb")
    nc.scalar.copy(_fr(v_sb[:]), ps[:])

    # att = softmax(q @ k.T / sqrt(300)) : [20, 20]
    ps = ps_t.tile([NN, NN], FP, tag="ps_t")
    for ki, (ko, ks) in enumerate(_ksl(DE)):
        _mm(nc, ps[:], qT[ki][:, :], kT[ki][:, :], start=(ki == 0), stop=(ki == 2))
    att_s = sm.tile([NN, NN], FP, tag="att_s")
    nc.scalar.activation(att_s[:], ps[:], AF.Identity, scale=float(1.0 / np.sqrt(DE)))
    mx = sm.tile([NN, 1], FP, tag="mx")
    nc.vector.tensor_reduce(mx[:], att_s[:], axis=mybir.AxisListType.X, op=ALU.max)
    negmx = sm.tile([NN, 1], FP, tag="negmx")
    nc.vector.tensor_scalar_mul(negmx[:], mx[:], -1.0)
    att_e = sm.tile([NN, NN], FP, tag="att_e")
    rs = sm.tile([NN, 1], FP, tag="rs")
    nc.scalar.activation(att_e[:], att_s[:], AF.Exp, bias=negmx[:, 0:1], accum_out=rs[:, 0:1])
    rr = sm.tile([NN, 1], FP, tag="rr")
    nc.vector.reciprocal(rr[:], rs[:])
    att_n = sm.tile([NN, NN], FP, tag="att_n")
    nc.vector.tensor_scalar_mul(att_n[:], att_e[:], rr[:, 0:1])

    # attT, AV = att @ v, node1col = AV^T @ (1/20)
    ps = ps_t.tile([NN, NN], FP, tag="ps_t")
    nc.tensor.transpose(ps[:], att_n[:], ident[0:NN, 0:NN])
    attT = sm.tile([NN, NN], FP, tag="attT")
    nc.scalar.copy(_fr(attT[:]), ps[:])
    ps = ps_t.tile([NN, DE], FP, tag="ps_t")
    _mm(nc, ps[:], attT[:, :], v_sb[:, :], start=True, stop=True)
    av_sb = sm.tile([NN, DE], FP, tag="av_sb")
    nc.scalar.copy(_fr(av_sb[:]), ps[:])

    n1c = sm.tile([128, 3], FP, tag="n1c")
    for mi, (mo, ms) in enumerate(_ksl(DE)):
        ps = ps_t.tile([ms, 8], FP, tag="ps_t")
        _mm(nc, ps[:], av_sb[:, mo:mo + ms], inv20[:, :], start=True, stop=True)
        nc.scalar.copy(_fr(n1c[0:ms, mi:mi + 1]), ps[:, 0:1])

    # node2 [1,300] = node1^T @ wo + bo ; ev = emb + bcast(node2)
    ps = ps_t.tile([1, DE], FP, tag="ps_t")
    for ki, (ko, ks) in enumerate(_ksl(DE)):
        _mm(nc, ps[:], n1c[0:ks, ki:ki + 1], wo[ki][:, :],
            start=(ki == 0), stop=(ki == 2))
    n2 = sm.tile([1, DE], FP, tag="n2")
    nc.vector.tensor_add(_fr(n2[:]), bo_row[:], ps[:])
    ps = ps_t.tile([NN, DE], FP, tag="ps_t")
    _mm(nc, ps[:], one_row[:, :], n2[:, :], start=True, stop=True)
    ev_sb = sm.tile([NN, DE], FP, tag="ev_sb")
    nc.vector.tensor_add(ev_sb[:], emb_sb[:], ps[:])

    # evT chunks [<=128, 20]
    evT = []
    for mi, (mo, ms) in enumerate(_ksl(DE)):
        ps = ps_t.tile([ms, NN], FP, tag="ps_t")
        nc.tensor.transpose(ps[:], ev_sb[:, mo:mo + ms], ident[0:NN, 0:NN])
        t = sm.tile([ms, NN], FP, tag=f"evT{mi}")
        nc.scalar.copy(_fr(t[:]), ps[:])
        evT.append(t)

    # adj_n = (d (x) d) * (adj + I)
    ah = sm.tile([NN, NN], FP, tag="ah")
    nc.gpsimd.tensor_add(ah[:], adj_sb[:], ident[0:NN, 0:NN])
    r20 = sm.tile([NN, 1], FP, tag="r20")
    nc.vector.tensor_reduce(r20[:], ah[:], axis=mybir.AxisListType.X, op=ALU.add)
    ir20 = sm.tile([NN, 1], FP, tag="ir20")
    nc.vector.reciprocal(ir20[:], r20[:])
    d20 = sm.tile([NN, 1], FP, tag="d20")
    nc.scalar.activation(d20[:], ir20[:], AF.Sqrt)
    ps = ps_t.tile([1, NN], FP, tag="ps_t")
    nc.tensor.transpose(ps[:], d20[:, 0:1], ident[0:NN, 0:NN])
    dT = sm.tile([1, NN], FP, tag="dT")
    nc.scalar.copy(_fr(dT[:]), ps[:])
    ps = ps_t.tile([NN, NN], FP, tag="ps_t")
    _mm(nc, ps[:], dT[:, :], dT[:, :], start=True, stop=True)
    adjn = sm.tile([NN, NN], FP, tag="adjn")
    nc.vector.tensor_mul(adjn[:], ah[:], ps[:])
    ps = ps_t.tile([NN, NN], FP, tag="ps_t")
    nc.tensor.transpose(ps[:], adjn[:], ident[0:NN, 0:NN])
    adjnT = sm.tile([NN, NN], FP, tag="adjnT")
    nc.scalar.copy(_fr(adjnT[:]), ps[:])

    # GCN layer 1: g1 = relu(adj_n @ (ev @ gc1_w))
    ps = ps_t.tile([NN, C], FP, tag="ps_t")
    for ki in range(3):
        _mm(nc, ps[:], evT[ki][:, :], gc1[ki][:, :], start=(ki == 0), stop=(ki == 2))
    t1 = sm.tile([NN, C], FP, tag="t1")
    nc.scalar.copy(_fr(t1[:]), ps[:])
    ps = ps_t.tile([NN, C], FP, tag="ps_t")
    _mm(nc, ps[:], adjnT[:, :], t1[:, :], start=True, stop=True)
    g1 = sm.tile([NN, C], FP, tag="g1")
    nc.scalar.activation(g1[:], ps[:], AF.Relu)

    g1T = []
    for mi, (mo, ms) in enumerate(_ksl(C)):
        ps = ps_t.tile([ms, NN], FP, tag="ps_t")
        nc.tensor.transpose(ps[:], g1[:, mo:mo + ms], ident[0:NN, 0:NN])
        t = sm.tile([ms, NN], FP, tag=f"g1T{mi}")
        nc.scalar.copy(_fr(t[:]), ps[:])
        g1T.append(t)

    ps = ps_t.tile([NN, C], FP, tag="ps_t")
    for ki in range(2):
        _mm(nc, ps[:], g1T[ki][:, :], gc2[ki][:, :], start=(ki == 0), stop=(ki == 1))
    t2 = sm.tile([NN, C], FP, tag="t2")
    nc.scalar.copy(_fr(t2[:]), ps[:])
    ps = ps_t.tile([NN, C], FP, tag="ps_t")
    _mm(nc, ps[:], adjnT[:, :], t2[:, :], start=True, stop=True)
    g2 = sm.tile([NN, C], FP, tag="g2")
    nc.scalar.activation(_fr(g2[:]), ps[:], AF.Relu)

    # reluG [128, 2] (column cb = relu(sum_m g2[m, 128cb:128cb+128]))
    reluG = sm.tile([128, 2], FP, tag="reluG")
    for cb in range(2):
        ps = ps_t.tile([128, 8], FP, tag="ps_t")
        _mm(nc, ps[:], g2[:, 128 * cb:128 * (cb + 1)], ones20[:, :], start=True, stop=True)
        nc.scalar.activation(_fr(reluG[:, cb:cb + 1]), ps[:, 0:1], AF.Relu)

    # fa [1, 256] = reluG^T @ Wa^T  (Wa^T = final_wT rows 256:512)
    ps = ps_t.tile([1, C], FP, tag="ps_t")
    for cb in range(2):
        _mm(nc, ps[:], reluG[:, cb:cb + 1], fwT[2 + cb][:, :],
            start=(cb == 0), stop=(cb == 1))
    fa = sm.tile([1, C], FP, tag="fa")
    nc.scalar.copy(_fr(fa[:]), ps[:])

    # WlgT [256, 256] = gw_w^T-contracted:  WlgT[c,o] = sum_k gw_w[k,c] Wl^T[k,o]
    WlgT = []
    for cb in range(2):
        ps = ps_t.tile([128, C], FP, tag="ps_t")
        for ki in range(2):
            _mm(nc, ps[:], gww[ki][:, 128 * cb:128 * (cb + 1)], fwT[ki][:, :],
                start=(ki == 0), stop=(ki == 1))
        t = sm.tile([128, C], FP, tag=f"WlgT{cb}")
        nc.scalar.copy(_fr(t[:]), ps[:])
        WlgT.append(t)

    # ---------------- per-batch pipeline ----------------
    x_cq = P["x"][:].rearrange("b (c q) -> b c q", c=C)       # [b, 256, 1024]
    x_pc = P["x"][:].rearrange("b (p c) -> b p c", c=C)       # [b, 1024, 256]
    out_cq = P["out"][:].rearrange("b (c q) -> b c q", c=C)
    r_jq = P["rscr"][:].rearrange("b (j q) -> b j q", j=MPHI)
    r_pj = P["rscr"][:].rearrange("b (p j) -> b p j", j=MPHI)

    for b in range(BPC):
        # loads
        xmat = []
        for j in range(2):
            t = big.tile([128, HW], FP, tag="xmat")
            nc.sync.dma_start(_fr(t[:]), _fr(x_cq[b, 128 * j:128 * (j + 1), :]))
            xmat.append(t)
        xv = []
        for t8 in range(8):
            t = xvp.tile([128, C + 8], FP, tag="xv")
            nc.sync.dma_start(_fr(t[:, 0:C]), _fr(x_pc[b, 128 * t8:128 * (t8 + 1), :]))
            nc.scalar.copy(_fr(t[:, C:C + 8]), onescol[:, 0:8])
            xv.append(t)

        # phi = phi_w @ xmat + phi_b ; R = relu(phi)
        ps_phi = ps_w.tile([MPHI, HW], FP, tag="ps_w")
        for ki in range(2):
            for nh in range(2):
                _mm(nc, ps_phi[:, 512 * nh:512 * (nh + 1)],
                    phiwT[ki][:, :], xmat[ki][:, 512 * nh:512 * (nh + 1)],
                    start=(ki == 0), stop=(ki == 1))
        R = med.tile([MPHI, HW], FP, tag="R")
        nc.scalar.activation(_fr(R[:]), ps_phi[:], AF.Relu, bias=phib[:, 0:1])
        nc.sync.dma_start(r_jq[b], R[:])

        # x_phi tiles [128,16] from scratch, PE-transpose into x_phiT_ext [17, 1024]
        xpT = med.tile([MPHI + 1, HW], FP, tag="xpT")
        for t8 in range(8):
            xp = sm.tile([128, MPHI], FP, tag="xp")
            nc.sync.dma_start(xp[:], r_pj[b, 128 * t8:128 * (t8 + 1), :])
            ps = ps_t.tile([MPHI, 128], FP, tag="ps_t")
            nc.tensor.transpose(ps[:], xp[:], ident[:, :])
            nc.scalar.copy(_fr(xpT[0:MPHI, 128 * t8:128 * (t8 + 1)]), ps[:])

        # g = glob_w @ mean(x) ; Dg entries
        xmean = sm.tile([128, 16], FP, tag="xmean")
        nc.vector.memset(xmean[:], 0.0)
        for ki in range(2):
            nc.vector.tensor_reduce(_fr(xmean[:, 8 * ki:8 * ki + 1]), xmat[ki][:],
                                    axis=mybir.AxisListType.X, op=ALU.add)
        ps_g = ps_t.tile([MPHI, 8], FP, tag="ps_t")
        for ki in range(2):
            _mm(nc, ps_g[:], globwT[ki][:, :], xmean[:, 8 * ki:8 * ki + 8],
                start=(ki == 0), stop=(ki == 1))
        sgm = sm.tile([MPHI, 1], FP, tag="sgm")
        nc.scalar.activation(sgm[:], ps_g[:, 0:1], AF.Sigmoid, scale=float(1.0 / HW))
        sm05 = sm.tile([MPHI, 1], FP, tag="sm05")
        nc.vector.tensor_scalar_add(sm05[:], sgm[:], -0.5)
        Dg = sm.tile([MPHI, MPHI], FP, tag="Dg")
        nc.vector.tensor_scalar(_fr(Dg[:]), ident[0:MPHI, 0:MPHI], sm05[:, 0:1], 0.5,
                                op0=ALU.mult, op1=ALU.add)

        # M_ext [17, 1024]: rows 0:16 = Dg @ R, row 16 = ones
        ps_m = ps_w.tile([MPHI, HW], FP, tag="ps_w")
        for nh in range(2):
            _mm(nc, ps_m[:, 512 * nh:512 * (nh + 1)], Dg[:, :],
                R[:, 512 * nh:512 * (nh + 1)], start=True, stop=True)
        Me = med.tile([MPHI + 1, HW], FP, tag="Me")
        nc.scalar.copy(_fr(Me[0:MPHI, :]), ps_m[:])
        nc.sync.dma_start(_fr(Me[MPHI:MPHI + 1, :]), _fr(P["crow"][0:1, :]))

        # -ub row: negMmax = -max_q M ; xpT row 16 = negMmax^T @ xpT[0:16]
        Mmax = sm.tile([MPHI, 1], FP, tag="Mmax")
        nc.vector.tensor_reduce(Mmax[:], Me[0:MPHI, :], axis=mybir.AxisListType.X,
                                op=ALU.max)
        negMm = sm.tile([MPHI, 1], FP, tag="negMm")
        nc.vector.tensor_scalar_mul(_fr(negMm[:]), Mmax[:], -1.0)
        ps_ub = ps_w.tile([1, HW], FP, tag="ps_w")
        for nh in range(2):
            _mm(nc, ps_ub[:, 512 * nh:512 * (nh + 1)], negMm[:, 0:1],
                xpT[0:MPHI, 512 * nh:512 * (nh + 1)], start=True, stop=True)
        nub = sm.tile([1, HW], FP, tag="nub")
        nc.scalar.copy(_fr(nub[:]), ps_ub[:])
        nc.sync.dma_start(_fr(xpT[MPHI:MPHI + 1, :]), _fr(nub[:]))

        # S^T tiles + exp -> E^T tiles [128, 1024]
        ET = []
        for t8 in range(8):
            ps_st = ps_w.tile([128, HW], FP, tag="ps_w")
            for nh in range(2):
                _mm(nc, ps_st[:, 512 * nh:512 * (nh + 1)],
                    Me[:, 128 * t8:128 * (t8 + 1)],
                    xpT[:, 512 * nh:512 * (nh + 1)], start=True, stop=True)
            et = etp.tile([128, HW], FP, tag="et")
            nc.scalar.activation(_fr(et[:]), ps_st[:], AF.Exp)
            ET.append(et)

        # per p-tile: EXV = E @ xv_ext (col 256 = D); spiral; transpose
        spT = [big.tile([128, HW], FP, tag="spT", name=f"spT{b}_{i}")
               for i in range(2)]
        for pt in range(8):
            ps_e = ps_x.tile([128, C + 8], FP, tag="ps_x")
            for k in range(8):
                _mm(nc, ps_e[:], ET[k][:, 128 * pt:128 * (pt + 1)], xv[k][:, :],
                    start=(k == 0), stop=(k == 7))
            negD = sm.tile([128, 1], FP, tag="negD")
            nc.vector.tensor_scalar_mul(negD[:], ps_e[:, C:C + 1], -1.0)
            nrd = sm.tile([128, 1], FP, tag="nrd")
            nc.vector.reciprocal(nrd[:], negD[:])
            spr = sm.tile([128, C], FP, tag="spr")
            nc.vector.scalar_tensor_tensor(spr[:], ps_e[:, 0:C], nrd[:, 0:1],
                                           xv[pt][:, 0:C], op0=ALU.mult, op1=ALU.add)
            for ch in range(2):
                ps = ps_t.tile([128, 128], FP, tag="ps_t")
                nc.tensor.transpose(ps[:], spr[:, 128 * ch:128 * (ch + 1)], ident[:, :])
                nc.scalar.copy(_fr(spT[ch][:, 128 * pt:128 * (pt + 1)]), ps[:])

        # sa = softmax over pixels of w_in . x
        ps_a = ps_w.tile([1, HW], FP, tag="ps_w")
        for ki in range(2):
            for nh in range(2):
                _mm(nc, ps_a[:, 512 * nh:512 * (nh + 1)], win[ki][:, :],
                    xmat[ki][:, 512 * nh:512 * (nh + 1)],
                    start=(ki == 0), stop=(ki == 1))
        ea = med.tile([1, HW], FP, tag="ea")
        sae = sm.tile([1, 1], FP, tag="sae")
        nc.scalar.activation(ea[:], ps_a[:], AF.Exp, accum_out=sae[:, 0:1])
        sar = sm.tile([1, 1], FP, tag="sar")
        nc.vector.reciprocal(sar[:], sae[:])
        sa = med.tile([1, HW], FP, tag="sa")
        nc.vector.tensor_scalar_mul(_fr(sa[:]), ea[:], sar[:, 0:1])

        # out[o,:] = relu(Wlg @ spiral^T + fa (x) sa + x)
        for ot in range(2):
            ps_o = ps_w.tile([128, HW], FP, tag="ps_w")
            for nh in range(2):
                sl = slice(512 * nh, 512 * (nh + 1))
                for ct in range(2):
                    _mm(nc, ps_o[:, sl], WlgT[ct][:, 128 * ot:128 * (ot + 1)],
                        spT[ct][:, sl], start=(ct == 0), stop=False)
                _mm(nc, ps_o[:, sl], fa[0:1, 128 * ot:128 * (ot + 1)], sa[0:1, sl],
                    start=False, stop=True)
            ob = big.tile([128, HW], FP, tag="ob", bufs=2)
            nc.vector.scalar_tensor_tensor(ob[:], ps_o[:], 1.0, xmat[ot][:],
                                           op0=ALU.mult, op1=ALU.add)
            nc.scalar.activation(ob[:], ob[:], AF.Relu)
            nc.sync.dma_start(out_cq[b, 128 * ot:128 * (ot + 1), :], ob[:])


def _const_col():
    cc = np.zeros((128, 16), np.float32)
    cc[:, 0] = 1.0
    cc[:, 8] = 1.0 / NN
    return cc


def _prep_shared(inputs):
    f = lambda k: np.ascontiguousarray(inputs[k], dtype=np.float32)
    shared = {
        "emb": f("emb"),
        "embTe": np.ascontiguousarray(
            np.vstack([f("emb").T, np.ones((1, NN), np.float32)])),
        "adj": f("adj"),
        "wq": f("wq"), "bq_col": f("bq").reshape(DE, 1),
        "wk": f("wk"), "bk_col": f("bk").reshape(DE, 1),
        "wve": np.ascontiguousarray(np.vstack([f("wv"), f("bv")[None, :]])),
        "wo": f("wo"), "bo_row": f("bo").reshape(1, DE),
        "phi_wT": np.ascontiguousarray(f("phi_w").T),
        "phi_b_col": f("phi_b").reshape(MPHI, 1),
        "glob_wT": np.ascontiguousarray(f("glob_w").T),
        "gc1_w": f("gc1_w"), "gc2_w": f("gc2_w"), "gw_w": f("gw_w"),
        "w_in_col": f("s2l_w")[:C].reshape(C, 1).copy(),
        "final_wT": np.ascontiguousarray(f("final_w").T),
        "const_col": _const_col(),
        "const_row": np.ones((1, HW), np.float32),
    }
    return shared


_NC_CACHE = {}


def kernel(**inputs):
    global LAST_EXEC_NS, LAST_RESULT
    if "nc" not in _NC_CACHE:
        _NC_CACHE["nc"] = _build_nc()
    nc = _NC_CACHE["nc"]

    x = np.ascontiguousarray(inputs["x"], dtype=np.float32)
    B = x.shape[0]
    shared = _prep_shared(inputs)
    in_maps = []
    for i in range(NCORES):
        m = dict(shared)
        m["x"] = np.ascontiguousarray(
            x[i * BPC:(i + 1) * BPC].reshape(BPC, C * HW))
        in_maps.append(m)

    trace = os.environ.get("KERNEL_TRACE", "0") == "1"
    res = run_bass_kernel_spmd(nc, in_maps, list(range(NCORES)), trace=trace)
    LAST_RESULT = res
    LAST_EXEC_NS = getattr(res, "exec_time_ns", None)

    out = np.empty((B, C, 32, 32), np.float32)
    for i in range(NCORES):
        out[i * BPC:(i + 1) * BPC] = res.results[i]["out"].reshape(BPC, C, 32, 32)
    return out
